# revision 1
# baseline (speedup 1.0000x reference)
"""Embedding lookup kernel for TRN2 (8 NeuronCores, SPMD data-parallel).

out[0, t, :] = W[:, idx[t]] + b   for t in [0, 32*8192)

Strategy: host precomputes table = W.T + b ([100000, 128] f32, 512B rows),
replicates it to all 8 cores; tokens sharded 32768/core.

Device path (plan B): the SWDGE `dma_gather` instruction gathers thousands
of 512B rows per instruction but takes int16 row indices (<= 32767), so the
host buckets each core's tokens by vocab window (idx >> 15; four 32768-row
windows). Per (window, chunk) the device runs one dma_gather (table window
-> SBUF, window-local indices) and one dma_scatter_add (SBUF -> out rows at
the original token positions; the output buffer is pre-zeroed by the
runtime, so += is plain assignment). Chunk capacities are static immediates;
real counts are a dense prefix and the -1 tail generates no descriptors
(HW-verified: tail -1s are skipped by both instructions, so padding moves
zero bytes and never races real rows).

Fallback (plan A, if a bucket overflows its static capacity — ~1e-70 for
uniform indices): plain indirect-DMA gather, 128 rows per instruction
(~3x slower, always correct).
"""

import numpy as np

import concourse.bacc as bacc
import concourse.mybir as mybir
import concourse.tile as tile
from concourse import bass
from concourse.bass_utils import run_bass_kernel_spmd

NCORES = 8
B, S = 32, 8192
TOKENS = B * S              # 262144
T = TOKENS // NCORES        # 32768 tokens per core
V = 100000
D = 128                     # embedding dim; 512 bytes per row (f32)

QW = 32767                  # vocab rows per window (int16 addressable - 1)
NQ = 4                      # windows; last covers V - 3*QW = 1699 rows
DEVW = 32768                # device window stride: QW real rows + 1 zero row
VDEV = NQ * DEVW            # 131072 rows in the device table
# (window, capacity) per chunk; per-window capacity is >=6 sigma above the
# binomial mean for uniform indices (10738/10738/10738/557).
CHUNKS = ([(0, 4096), (0, 4096), (0, 3072)]
          + [(1, 4096), (1, 4096), (1, 3072)]
          + [(2, 4096), (2, 4096), (2, 3072)]
          + [(3, 1024)])
NCH = len(CHUNKS)
QCAP = [4096 + 4096 + 3072] * 3 + [1024]
CAPMAX = 4096

_compiled = {}


def _repeat_chunks(repeat):
    for _ in range(repeat):
        yield from enumerate(CHUNKS)


def _build_plan_b(repeat=1):
    # repeat>1 replicates the body for repeat-slope timing (output values
    # then accumulate and are wrong; timing only).
    nc = bacc.Bacc("TRN2", target_bir_lowering=False, debug=False)
    idx16_d = nc.dram_tensor("idx16", [NCH, 128, CAPMAX // 16], mybir.dt.int16,
                             kind="ExternalInput").ap()
    pos16_d = nc.dram_tensor("pos16", [NCH, 128, CAPMAX // 16], mybir.dt.int16,
                             kind="ExternalInput").ap()
    tab_d = nc.dram_tensor("tab", [VDEV, D], mybir.dt.float32,
                           kind="ExternalInput").ap()
    out_d = nc.dram_tensor("out", [T, D], mybir.dt.float32,
                           kind="ExternalOutput").ap()

    with tile.TileContext(nc) as tc:
        with tc.tile_pool(name="idxp", bufs=4) as ip, \
             tc.tile_pool(name="data", bufs=3) as dp:
            for ch, (q, cap) in _repeat_chunks(repeat):
                it = ip.tile([128, cap // 16], mybir.dt.int16, tag="it")
                nc.sync.dma_start(out=it[:], in_=idx16_d[ch, :, :cap // 16])
                pt = ip.tile([128, cap // 16], mybir.dt.int16, tag="pt")
                nc.sync.dma_start(out=pt[:], in_=pos16_d[ch, :, :cap // 16])
                dt_ = dp.tile([128, cap], mybir.dt.float32)
                dt3 = dt_[:].rearrange("p (s e) -> p s e", e=D)
                nc.gpsimd.dma_gather(
                    dt3, tab_d[q * DEVW:(q + 1) * DEVW, :], it[:],
                    num_idxs=cap, num_idxs_reg=cap, elem_size=D,
                    single_packet=False)
                nc.gpsimd.dma_scatter_add(
                    out_d[:], dt3, pt[:],
                    num_idxs=cap, num_idxs_reg=cap, elem_size=D,
                    single_packet=False)
    nc.compile()
    return nc


def _build_plan_a():
    G = 8
    NGATH = T // 128
    NGRP = T // (128 * G)
    nc = bacc.Bacc("TRN2", target_bir_lowering=False, debug=False)
    idx_d = nc.dram_tensor("idx", [128, NGATH], mybir.dt.int32,
                           kind="ExternalInput").ap()
    tab_d = nc.dram_tensor("tab", [V, D], mybir.dt.float32,
                           kind="ExternalInput").ap()
    out_d = nc.dram_tensor("out", [T, D], mybir.dt.float32,
                           kind="ExternalOutput").ap()
    with tile.TileContext(nc) as tc:
        with tc.tile_pool(name="data", bufs=3) as dp, \
             tc.tile_pool(name="idxp", bufs=1) as ip:
            it = ip.tile([128, NGATH], mybir.dt.int32)
            nc.sync.dma_start(out=it[:], in_=idx_d[:])
            for c in range(NGRP):
                dt_ = dp.tile([128, G * D], mybir.dt.float32)
                for g in range(G):
                    nc.gpsimd.indirect_dma_start(
                        out=dt_[:, g * D:(g + 1) * D], out_offset=None,
                        in_=tab_d[:],
                        in_offset=bass.IndirectOffsetOnAxis(
                            ap=it[:, c * G + g:c * G + g + 1], axis=0),
                    )
                dst = out_d[c * G * 128:(c + 1) * G * 128, :] \
                    .rearrange("(g p) d -> p g d", p=128)
                nc.sync.dma_start(
                    out=dst, in_=dt_[:].rearrange("p (g d) -> p g d", g=G))
    nc.compile()
    return nc


def _get_nc(plan):
    if plan not in _compiled:
        _compiled[plan] = _build_plan_b() if plan == "b" else _build_plan_a()
    return _compiled[plan]


def _wrap16(arr):
    # slot i -> partition i % 16, column i // 16; replicated to 128 partitions
    w = arr.reshape(-1, 16).T            # [16, n/16]
    return np.ascontiguousarray(np.tile(w, (8, 1)))


def _pack_core_plan_b(idx):
    """idx: [T] int32 for one core -> (idx16, pos16) or None on overflow.

    Every entry is valid (the SWDGE ring corrupts when an instruction emits
    fewer descriptors than num_idxs). Real entries form a dense prefix;
    gather pads fetch the window's zero row (local index QW) and scatter
    pads add those zeros to rows owned by a DISTANT chunk — an exact no-op
    that cannot race the pad's own instruction (disjoint rows) nor
    concurrently-running scatters (distant chunks never overlap in time).
    """
    q = np.minimum(idx // QW, NQ - 1).astype(np.int64)
    counts = np.bincount(q, minlength=NQ)
    if (counts > np.asarray(QCAP)).any():
        return None
    order = np.argsort(q, kind="stable").astype(np.int64)
    bounds = np.concatenate([[0], np.cumsum(counts)])

    idx16 = np.full((NCH, CAPMAX), QW, np.int16)    # pad: window zero row
    pos16 = np.zeros((NCH, CAPMAX), np.int16)
    taken = [0, 0, 0, 0]
    reals = []
    for ch, (qq, cap) in enumerate(CHUNKS):
        s = bounds[qq] + taken[qq]
        n = min(int(counts[qq]) - taken[qq], cap)
        taken[qq] += n
        toks = order[s:s + n]
        idx16[ch, :n] = (idx[toks] - qq * QW).astype(np.int16)
        pos16[ch, :n] = toks.astype(np.int16)
        reals.append((n, toks))
    for ch, (qq, cap) in enumerate(CHUNKS):
        n = reals[ch][0]
        if n < cap:
            donor = reals[(ch + NCH // 2) % NCH][1]
            if donor.size == 0:
                donor = reals[(ch + NCH // 2 + 1) % NCH][1]
            pad = np.resize(donor, cap - n)
            pos16[ch, n:cap] = pad.astype(np.int16)
    idx16 = np.stack([_wrap16(idx16[ch]) for ch in range(NCH)])
    pos16 = np.stack([_wrap16(pos16[ch]) for ch in range(NCH)])
    return idx16, pos16


def _make_dev_table(table):
    """[V, D] -> [VDEV, D]: four 32768-row windows of QW vocab rows (last
    window short) each followed by zero rows (the pad target)."""
    tdev = np.zeros((VDEV, D), np.float32)
    for q in range(NQ):
        lo = q * QW
        hi = min(lo + QW, V)
        tdev[q * DEVW:q * DEVW + (hi - lo)] = table[lo:hi]
    return tdev


def _make_in_maps(X, W, b):
    X = np.asarray(X)
    W = np.asarray(W, dtype=np.float32)
    b = np.asarray(b, dtype=np.float32)

    idx = np.ascontiguousarray(X.reshape(-1).astype(np.int32))
    table = np.ascontiguousarray(W.T) + b[None, :]

    packs = [_pack_core_plan_b(idx[c * T:(c + 1) * T]) for c in range(NCORES)]
    if all(p is not None for p in packs):
        tdev = _make_dev_table(table)
        return "b", [
            {"idx16": p[0], "pos16": p[1], "tab": tdev}
            for p in packs
        ]
    # overflow (pathological index distribution): plan A fallback
    NGATH = T // 128
    return "a", [
        {"idx": np.ascontiguousarray(
            idx[c * T:(c + 1) * T].reshape(NGATH, 128).T), "tab": table}
        for c in range(NCORES)
    ]


def _gather_out(res):
    out = np.concatenate(
        [res.results[c]["out"] for c in range(NCORES)], axis=0
    )
    return out.reshape(1, TOKENS, D)


def kernel(X, W, b):
    plan, in_maps = _make_in_maps(X, W, b)
    res = run_bass_kernel_spmd(_get_nc(plan), in_maps, list(range(NCORES)))
    return _gather_out(res)

